# revision 16
# baseline (speedup 1.0000x reference)
"""Trainium2 Bass kernel for nn_EnergyEwald (gnn_message_passing) — v2.

Same sharding as v1 (8 cores, atoms + pairs dealt round-robin, host combine),
but the reciprocal-space device work is restructured around the angle
addition theorem:

    e^{2pi i (kx ux + ky uy + kz uz)} = e^{2pi i theta_xy} * e^{2pi i theta_z}

Per atom the device only evaluates sin/cos of the 127 distinct phases
(6 pure-z phases kz*uz plus 121 xy-pair phases kx*ux+ky*uy) instead of all
510 half-lattice phases.  The per-molecule segment sum AND the angle
addition are then fused into one tensor-engine contraction per 128-atom
chunk:

    out[r, c] = sum_a lhsT[a, r] * cs[a, c]

with lhsT = [q | q*sin(th_z) | q*cos(th_z)] (13 columns) and cs the per-atom
[cos | sin] phase table.  Atoms are laid out in 2 chunks of 128 per molecule
(fixed 256-atom slots), so each molecule's contraction accumulates in PSUM
over exactly 2 chunks; 4 molecules share one PSUM tile at partition offsets
0/32/64/96.  The host recombines Re/Im parts (O(M*K)) and applies the tail
math in float64.

All matmuls run in fp32r (11-bit mantissa, 4x fp32 PE rate); the phase
matmul uses an exact u = u_hi + u_lo split for full precision.  Real space
is unchanged from v1: host ships d and qi*qj, device does
erfc(sqrt(a)d)/d - cutoff, masks, and column-sums via a ones-matmul.
"""

import math
import os
import sys
from contextlib import ExitStack

import numpy as np

for _p in ("/opt/trn_rl_repo", "/root/.axon_site/_ro/trn_rl_repo"):
    if os.path.isdir(_p) and _p not in sys.path:
        sys.path.insert(0, _p)

import concourse.tile as tile  # noqa: E402
from concourse import bacc, bass_utils, mybir  # noqa: E402

KE = 14.3996
ALPHA = 0.3
CUTOFF = 10.0
SQA = math.sqrt(ALPHA)
FCUT = math.erfc(SQA * CUTOFF) / CUTOFF
TWO_PI = 2.0 * math.pi
MAGIC = 12582912.0  # 1.5 * 2**23: float32 round-to-nearest-integer trick

N_CORES = 8
N_ATOMS = 100000
N_PAIRS = 6400000
N_MOL = 64

APC = N_ATOMS // N_CORES          # atoms per core = 12500
MSLOT = 256                       # atom slots per molecule per core
NCH = N_MOL * MSLOT // 128        # 128 chunks of 128 atom slots
NGR = NCH // 8                    # 16 groups (4 molecules / 8 chunks each)

NZ = 6                            # pure-z phases (kz = zs[0..5])
NXY = 121                         # xy-pair phases
PH = NZ + NXY                     # 127 phases per atom

CT = 1024                         # real-space tile columns
NRT = 7                           # real-space tiles
CCOLS = NRT * CT                  # 7168 columns of 128 pairs per core
PAD_X = 50.0                      # pad pair distance -> masked out
RC_GROUPS = (1, 3, 5, 7, 9, 11, 13)  # groups that emit a real-space tile

F32 = mybir.dt.float32
F32R = mybir.dt.float32r
BF16 = mybir.dt.bfloat16

_PROG_CACHE = {}


def to_f32r(a):
    """Round float32 data to the fp32r grid (e8m11: low 12 mantissa bits
    zero, round-to-nearest-even)."""
    b = np.ascontiguousarray(a, np.float32).view(np.uint32).copy()
    lo = b & np.uint32(0xFFF)
    hi = b & ~np.uint32(0xFFF)
    rup = (lo > 0x800) | ((lo == 0x800) & (((hi >> 12) & 1) == 1))
    return (hi + (rup.astype(np.uint32) << 12)).view(np.float32)


def _build_program(reps=1):
    AluOp = mybir.AluOpType
    AF = mybir.ActivationFunctionType

    nc = bacc.Bacc("TRN2", target_bir_lowering=False, debug=False,
                   num_devices=N_CORES)

    def din(name, shape, dt=F32):
        return nc.dram_tensor(name, shape, dt, kind="ExternalInput").ap()

    def dout(name, shape):
        return nc.dram_tensor(name, shape, F32, kind="ExternalOutput").ap()

    u_hi = din("u_hi", [3, NCH * 128], F32R)   # fractional coords, hi part
    u_lo = din("u_lo", [3, NCH * 128], F32R)   # fractional coords, lo part
    kmat = din("kmat", [3, 256], F32R)         # phase k-matrix, padded
    one_t = din("one_t", [128, 1], F32R)       # ones column
    q_t = din("q_t", [128, NCH], F32)          # q[slot] by (partition, chunk)
    ds = din("ds", [128, CCOLS])               # pair distance d
    qq = din("qq", [128, CCOLS])               # q[i]*q[j] per pair

    o_k = dout("o_k", [NGR, 128, 256])         # factored (mol, kz) x xy sums
    o_cs = dout("o_cs", [1, CCOLS])            # per-column pair-pot sums

    with tile.TileContext(nc, trace_sim=False) as tc, ExitStack() as ctx:
        pers = ctx.enter_context(tc.tile_pool(name="pers", bufs=1))
        io = ctx.enter_context(tc.tile_pool(name="io", bufs=3))
        tmp = ctx.enter_context(tc.tile_pool(name="tmp", bufs=2))
        rtmp = ctx.enter_context(tc.tile_pool(name="rtmp", bufs=2))
        ps_t = ctx.enter_context(tc.tile_pool(name="ps_t", bufs=2,
                                              space="PSUM"))
        ps_acc = ctx.enter_context(tc.tile_pool(name="ps_acc", bufs=2,
                                                space="PSUM"))
        ps_cs = ctx.enter_context(tc.tile_pool(name="ps_cs", bufs=1,
                                               space="PSUM"))

        # persistent SBUF
        km_sb = pers.tile([3, 256], F32R)
        q_sb = pers.tile([128, NCH], F32)
        ones = pers.tile([128, 1], F32R)
        halfpi = pers.tile([128, 1], F32)
        negq = pers.tile([128, 1], F32)

        nc.vector.memset(halfpi[:], math.pi / 2)
        nc.vector.memset(negq[:], -0.25)
        nc.sync.dma_start(ones[:], one_t[:])
        nc.sync.dma_start(km_sb[:], kmat[:])
        nc.sync.dma_start(q_sb[:], q_t[:])

        def _emit_rc_tile(i):
            # one real-space tile: stream d/qq, pair potential, column reduce
            sl = slice(i * CT, (i + 1) * CT)
            dt_ = io.tile([128, CT], F32, tag="dt")
            nc.sync.dma_start(dt_[:], ds[:, sl])
            qqt = io.tile([128, CT], F32, tag="qq")
            nc.sync.dma_start(qqt[:], qq[:, sl])
            ri = rtmp.tile([128, CT], F32, tag="ri")
            nc.vector.reciprocal(ri[:], dt_[:])
            e = rtmp.tile([128, CT], F32, tag="e")
            nc.scalar.activation(e[:], dt_[:], AF.Erf, scale=-SQA)
            f = rtmp.tile([128, CT], F32, tag="f")
            nc.vector.scalar_tensor_tensor(f[:], e[:], 1.0, ri[:],
                                           AluOp.add, AluOp.mult)
            # cutoff: f(d) strictly decreasing -> relu(f - FCUT) masks d>CUT
            p = rtmp.tile([128, CT], F32, tag="p")
            nc.vector.tensor_scalar(p[:], f[:], FCUT, 0.0,
                                    AluOp.subtract, AluOp.max)
            pot = rtmp.tile([128, CT], F32R, tag="pot")
            nc.gpsimd.tensor_tensor(pot[:], p[:], qqt[:], AluOp.mult)
            for j in range(CT // 512):
                cps = ps_cs.tile([1, 512], F32, tag="cs")
                nc.tensor.matmul(cps[:], ones[:],
                                 pot[:, j * 512:(j + 1) * 512],
                                 start=True, stop=True)
                cs_sb = io.tile([1, 512], F32, tag="cso")
                nc.vector.tensor_copy(cs_sb[:], cps[:])
                lo = i * CT + j * 512
                nc.sync.dma_start(o_cs[0:1, lo:lo + 512], cs_sb[:])

        def _emit_once(rep):
            rc_next = 0
            for g in range(NGR):
                if g in RC_GROUPS:
                    _emit_rc_tile(rc_next)
                    rc_next += 1
                c0 = 8 * g                      # first chunk of the group
                uh = io.tile([3, 1024], F32R, tag="uh")
                nc.sync.dma_start(uh[:], u_hi[:, c0 * 128:(c0 + 8) * 128])
                ul = io.tile([3, 1024], F32R, tag="ul")
                nc.sync.dma_start(ul[:], u_lo[:, c0 * 128:(c0 + 8) * 128])

                # phase matmuls: t[atom, phase] per chunk, 4 chunks per
                # PSUM tile, exact u = u_hi + u_lo accumulation
                tts = []
                for qd in range(2):
                    ttq = ps_t.tile([128, 1024], F32, tag="tt")
                    for j in range(4):
                        asl = slice((4 * qd + j) * 128,
                                    (4 * qd + j + 1) * 128)
                        osl = slice(256 * j, 256 * (j + 1))
                        nc.tensor.matmul(ttq[:, osl], uh[:, asl], km_sb[:],
                                         start=True, stop=False)
                        nc.tensor.matmul(ttq[:, osl], ul[:, asl], km_sb[:],
                                         start=False, stop=True)
                    tts.append(ttq)

                # range reduce t mod 1.  Per-chunk 256-col layout in wga:
                # [ |g| (127) pad | w = 0.25-g (127) pad ]; one Sin with
                # scale=-2pi bias=pi/2 turns that into [cos | sin].
                rnd = tmp.tile([128, 1016], F32, tag="rnd")
                wga = tmp.tile([128, 2048], F32, tag="wga")
                # bf16 phase table: the accumulation matmul runs in bf16
                # (fp32r matmuls cannot write PSUM at partition offsets)
                cs = tmp.tile([128, 2048], BF16, tag="cs")
                if rep == 0 and g < 2:
                    # pool-rotation bootstrap: define the pad columns once
                    nc.vector.memset(wga[:], 0.0)
                for qd in range(2):
                    tt_ph = tts[qd][:].rearrange("p (j c) -> p j c",
                                                 j=4)[:, :, 0:PH]
                    ro = rnd[:, 508 * qd:508 * qd + 508] \
                        .rearrange("p (j c) -> p j c", j=4)
                    nc.vector.tensor_scalar(ro, tt_ph, MAGIC, MAGIC,
                                            AluOp.add, AluOp.subtract)
                    wo = wga[:, 1024 * qd:1024 * qd + 1024] \
                        .rearrange("p (j c) -> p j c", j=4)[:, :, 128:255]
                    nc.vector.scalar_tensor_tensor(wo, ro, 0.25, tt_ph,
                                                   AluOp.add, AluOp.subtract)
                wga_g = wga[:].rearrange("p (j c) -> p j c", j=8)
                nc.scalar.activation(wga_g[:, :, 0:127],
                                     wga_g[:, :, 128:255], AF.Abs,
                                     bias=negq[:])
                nc.scalar.activation(cs[:], wga[:], AF.Sin,
                                     scale=-TWO_PI, bias=halfpi[:])

                # lhsT = [q | q*sin_z | q*cos_z] per chunk, 16-col slots
                lhq = tmp.tile([128, 128], BF16, tag="lhq")
                qb = q_sb[:, c0:c0 + 8]
                qbb = qb.unsqueeze(2).broadcast_to([128, 8, 6])
                lhq_s = lhq[:].rearrange("p (j c) -> p j c", j=8)
                cs_g = cs[:].rearrange("p (j c) -> p j c", j=8)
                nc.vector.tensor_copy(lhq_s[:, :, 0:1], qb.unsqueeze(2))
                # cols 1:7 <- q*sin_z (cs 128:134), 7:13 <- q*cos_z (cs 0:6)
                nc.vector.tensor_tensor(lhq_s[:, :, 1:7],
                                        cs_g[:, :, 128:134], qbb, AluOp.mult)
                nc.vector.tensor_tensor(lhq_s[:, :, 7:13],
                                        cs_g[:, :, 0:6], qbb, AluOp.mult)

                # per-molecule accumulation: mol = jj//2, psum col 32*(jj//2)
                acc = ps_acc.tile([128, 256], F32, tag="acc")
                if rep == 0 and g < 2:
                    nc.vector.memset(acc[:], 0.0)
                for jj in range(8):
                    m4 = jj // 2
                    nc.tensor.matmul(acc[32 * m4:32 * m4 + 13, :],
                                     lhq[:, 16 * jj:16 * jj + 13],
                                     cs[:, 256 * jj:256 * jj + 256],
                                     start=(jj % 2 == 0), stop=(jj % 2 == 1),
                                     tile_position=(0, 32 * m4),
                                     skip_group_check=True)
                ko = io.tile([128, 256], F32, tag="ko")
                nc.vector.tensor_copy(ko[:], acc[:])
                nc.sync.dma_start(o_k[g], ko[:])

        for rep in range(reps):
            _emit_once(rep)

    nc.compile()
    return nc


def _get_program(nsets=1, reps=1):
    # nsets is accepted (and ignored) for test-harness signature parity
    if reps not in _PROG_CACHE:
        _PROG_CACHE[reps] = _build_program(reps)
    return _PROG_CACHE[reps]


def _kvec_structure(kvecs):
    """Half-set + (xy, z) factorization of the kvec lattice.  Returns None
    if the kvec set does not have the required structure."""
    kv = np.asarray(kvecs)
    ki = np.round(kv).astype(np.int64)
    if not np.allclose(kv, ki, atol=1e-5):
        return None
    keyset = {tuple(v) for v in ki}
    if any(tuple(-v) not in keyset for v in ki):
        return None
    if any((v == 0).all() for v in ki):
        return None
    # half set: kz > 0, or kz == 0 and (ky > 0 or (ky == 0 and kx > 0))
    half = np.array([(z > 0) or (z == 0 and (y > 0 or (y == 0 and x > 0)))
                     for x, y, z in ki])
    sel = np.where(half)[0]
    if sel.size * 2 != len(ki):
        return None
    hk = ki[sel]
    zs = sorted({int(z) for _, _, z in hk if z > 0})
    xy = sorted({(int(x), int(y)) for x, y, _ in hk})
    if len(zs) > NZ or len(xy) > NXY:
        return None
    zpos = {z: i for i, z in enumerate(zs)}
    xypos = {p: i for i, p in enumerate(xy)}
    return sel, hk, zs, xy, zpos, xypos


def prepare(inputs):
    """Host prep: returns (nc, in_maps, combine_fn)."""
    q = np.asarray(inputs["partial_charges"], np.float32)[:, 0]
    Rij = np.asarray(inputs["Rij"], np.float32)
    R = np.asarray(inputs["R"], np.float32)
    cell = np.asarray(inputs["cell"], np.float32)
    kvecs = np.asarray(inputs["kvecs"], np.float32)
    idx_m = np.asarray(inputs["idx_m"]).astype(np.int64)
    idx_i = np.asarray(inputs["idx_i"]).astype(np.int64)
    idx_j = np.asarray(inputs["idx_j"]).astype(np.int64)

    st = _kvec_structure(kvecs)
    assert st is not None, "kvec set lacks the +-/xy-z factored structure"
    sel, hk, zs, xy, zpos, xypos = st

    nc = _get_program()

    # ---------- host prep: reciprocal space ----------
    invc = np.linalg.inv(cell.astype(np.float64))
    u_all = np.einsum("ae,aed->ad", R, invc[idx_m]).astype(np.float32)

    km_np = np.zeros((3, 256), np.float32)
    for i, z in enumerate(zs):
        km_np[2, i] = z
    for i, (x, y) in enumerate(xy):
        km_np[0, NZ + i] = x
        km_np[1, NZ + i] = y

    # ---------- host prep: real space ----------
    mol_pair = idx_m[idx_i].astype(np.int32)
    qq_pair = q[idx_i] * q[idx_j]
    d_pair = np.sqrt(np.einsum("pd,pd->p", Rij, Rij)).astype(np.float32)
    order = np.argsort(mol_pair, kind="stable")
    d_s = d_pair[order]
    qq_s = qq_pair[order]
    counts = np.bincount(mol_pair, minlength=N_MOL)
    starts = np.concatenate(([0], np.cumsum(counts)))

    acounts = np.bincount(idx_m, minlength=N_MOL)
    astarts = np.concatenate(([0], np.cumsum(acounts)))

    in_maps = []
    colmols = []
    SLOTS = CCOLS * 128
    for c in range(N_CORES):
        gidx = np.full(SLOTS, -1, np.int64)   # [p, col] flattened p*CCOLS+col
        colmol = np.full(CCOLS, -1, np.int32)
        col0 = 0
        for m in range(N_MOL):
            n = counts[m]
            share = (n + N_CORES - 1) // N_CORES
            lo = starts[m] + c * share
            hi = min(starts[m] + n, lo + share)
            ncm = max(hi - lo, 0)
            if ncm == 0:
                continue
            ncols = (ncm + 127) // 128
            js = np.arange(ncm)
            gidx[(js % 128) * CCOLS + col0 + js // 128] = lo + js
            colmol[col0:col0 + ncols] = m
            col0 += ncols
        assert col0 <= CCOLS, f"column overflow: {col0} > {CCOLS}"
        valid = gidx >= 0
        gv = gidx[valid]

        def fill(src, pad):
            a = np.full(SLOTS, pad, np.float32)
            a[valid] = src[gv]
            return a.reshape(128, CCOLS)

        # atoms for this core into fixed 256-slot molecule bins (atoms are
        # sorted by molecule, so slicing each molecule's range round-robin
        # deals ~195 +- 15 atoms per (core, molecule) — well under 256)
        u_core = np.zeros((N_MOL * MSLOT, 3), np.float32)
        q_core = np.zeros(N_MOL * MSLOT, np.float32)
        for m in range(N_MOL):
            n = acounts[m]
            ids = np.arange(astarts[m] + c, astarts[m] + n, N_CORES)
            nm = ids.size
            assert nm <= MSLOT, f"molecule {m} overflows its slot: {nm}"
            u_core[m * MSLOT:m * MSLOT + nm] = u_all[ids]
            q_core[m * MSLOT:m * MSLOT + nm] = q[ids]

        # [slot, 3] -> [3, slot] with slot = chunk*128 + partition
        u_hi_core = to_f32r(u_core)
        u_lo_core = to_f32r(u_core - u_hi_core)
        u_hi_np = np.ascontiguousarray(u_hi_core.T)
        u_lo_np = np.ascontiguousarray(u_lo_core.T)
        # q_t[p, ch] = q_core[ch*128 + p]
        q_t_np = np.ascontiguousarray(
            to_f32r(q_core).reshape(NCH, 128).T)

        in_maps.append({
            "u_hi": u_hi_np,
            "u_lo": u_lo_np,
            "kmat": km_np,
            "one_t": np.ones((128, 1), np.float32),
            "q_t": q_t_np,
            "ds": fill(d_s, PAD_X),
            "qq": fill(qq_s, 0.0),
        })
        colmols.append(colmol)

    self_q2_host = np.bincount(idx_m, weights=(q.astype(np.float64) ** 2),
                               minlength=N_MOL)

    # index maps for the factored recombination
    nkh = hk.shape[0]
    xyi = np.array([xypos[(int(x), int(y))] for x, y, _ in hk])
    kzv = hk[:, 2]

    def combine(results):
        ok = np.zeros((NGR, 128, 256), np.float64)
        y_real = np.zeros(64, np.float64)
        for c in range(N_CORES):
            out = results[c]
            ok += out["o_k"]
            cs = out["o_cs"][0]
            cm = colmols[c]
            used = cm >= 0
            y_real += np.bincount(cm[used], weights=cs[used], minlength=64)

        # per-molecule factored blocks: B[mol, r, c]
        B = ok.reshape(NGR, 4, 32, 256)[:, :, 0:13, :] \
              .reshape(N_MOL, 13, 256)
        A0c = B[:, 0, 6:127]          # [M, 121]  sum q cos(th_xy)
        A0s = B[:, 0, 134:255]        # [M, 121]  sum q sin(th_xy)
        Ssc = B[:, 1:7, 6:127]        # [M, 6, 121]  sum q sin(th_z) cos(xy)
        Sss = B[:, 1:7, 134:255]
        Scc = B[:, 7:13, 6:127]
        Scs = B[:, 7:13, 134:255]

        q_real = np.zeros((N_MOL, nkh), np.float64)
        q_imag = np.zeros((N_MOL, nkh), np.float64)
        z_mask = kzv > 0
        zi = np.array([zpos[int(z)] if z > 0 else 0 for z in kzv])
        pz = xyi[z_mask]
        zz = zi[z_mask]
        q_real[:, z_mask] = Scc[:, zz, pz] - Sss[:, zz, pz]
        q_imag[:, z_mask] = Scs[:, zz, pz] + Ssc[:, zz, pz]
        q_real[:, ~z_mask] = A0c[:, xyi[~z_mask]]
        q_imag[:, ~z_mask] = A0s[:, xyi[~z_mask]]

        # O(M*K) tail math (float64 on host, cast at the end)
        recip = TWO_PI * np.transpose(invc, (0, 2, 1))     # [M,3,3]
        v_box = np.abs(np.linalg.det(cell.astype(np.float64)))
        prefactor = TWO_PI / v_box
        kv_m = np.einsum("kd,mde->mke", hk.astype(np.float64), recip)
        k_sq = np.sum(kv_m ** 2, axis=2)                   # [M,Kh]
        q_gauss = np.exp(-0.25 * k_sq / ALPHA)
        q_dens = q_real ** 2 + q_imag ** 2
        y_ewald = prefactor * np.sum(2.0 * q_dens * q_gauss / k_sq, axis=1)
        self_int = math.sqrt(ALPHA / math.pi) * self_q2_host
        y = 0.5 * KE * y_real + KE * (y_ewald - self_int)
        return y.astype(np.float32)

    return nc, in_maps, combine


def kernel(**inputs):
    nc, in_maps, combine = prepare(inputs)
    res = bass_utils.run_bass_kernel_spmd(nc, in_maps,
                                          core_ids=list(range(N_CORES)))
    return combine(res.results)


# revision 22
# speedup vs baseline: 4.0184x; 4.0184x over previous
"""Trainium2 Bass kernel for nn_EnergyEwald (gnn_message_passing).

Strategy (8 NeuronCores, SPMD, host combines partials):
  * Real space: pairs are sorted by molecule-of-i on the host, dealt to the 8
    cores, and laid out column-major so every 128-pair SBUF column belongs to
    one molecule.  The host precomputes the pair distance d = |Rij| and
    qi*qj; the device streams d + qq, computes erfc(sqrt(a) d)/d with the
    cutoff mask, reduces each column with a ones-matmul on the tensor engine
    (fp32r, 4x the fp32 matmul rate), and returns per-column sums; the host
    segment-sums columns into the 64 molecules.
  * Reciprocal space: per-atom fractional phases u = R @ inv(cell_mol) are
    computed on the host (tiny O(A) work); the device computes the phase
    matrix t = kvec . u with the tensor engine (atoms sharded across cores),
    range-reduces t mod 1 with the magic-number rounding trick on the vector
    engine, evaluates sin/cos on the scalar engine (Sin table is only valid
    on [-pi, pi]), and segment-sums q*cos / q*sin into [mol, kvec]
    accumulators in PSUM via a q-scaled one-hot matmul.  +-k symmetry of the
    kvec lattice halves the transcendental work (checked at runtime, with a
    compiled fallback for asymmetric kvec sets).
  * All matmuls run in fp32r (fp32 with the mantissa rounded to 11 bits,
    which the PE processes at 4x the fp32 rate).  The phase matmul needs
    more than 11 mantissa bits of u, so u is split u = u_hi + u_lo (both
    exactly representable in fp32r) and accumulated with two matmuls.
  * Host combines the 8 cores' partial sums (the "all-reduce") and applies
    the O(M*K) tail math (q_gauss, k_sq, prefactors, self-interaction).
"""

import math
import os
import sys
from contextlib import ExitStack

import numpy as np

for _p in ("/opt/trn_rl_repo", "/root/.axon_site/_ro/trn_rl_repo"):
    if os.path.isdir(_p) and _p not in sys.path:
        sys.path.insert(0, _p)

import concourse.tile as tile  # noqa: E402
from concourse import bacc, bass_utils, mybir  # noqa: E402

KE = 14.3996
ALPHA = 0.3
CUTOFF = 10.0
SQA = math.sqrt(ALPHA)
FCUT = math.erfc(SQA * CUTOFF) / CUTOFF
TWO_PI = 2.0 * math.pi
MAGIC = 12582912.0  # 1.5 * 2**23: float32 round-to-nearest-integer trick

N_CORES = 8
N_ATOMS = 100000
N_PAIRS = 6400000
N_MOL = 64

APC = N_ATOMS // N_CORES          # atoms per core = 12500
ACH = 98                          # 128-atom chunks per core
APAD = ACH * 128                  # 12544
NSUP = ACH // 2                   # super-chunks of 256 atoms

CT = 1024                         # real-space tile columns
NRT = 7                           # real-space tiles
CCOLS = NRT * CT                  # 7168 columns of 128 pairs per core
PAD_X = 50.0                      # pad pair distance -> masked out

F32 = mybir.dt.float32
F32R = mybir.dt.float32r
F16 = mybir.dt.float16

_PROG_CACHE = {}


def to_f32r(a):
    """Round float32 data to the fp32r grid (e8m11: low 12 mantissa bits
    zero, round-to-nearest-even)."""
    b = np.ascontiguousarray(a, np.float32).view(np.uint32).copy()
    lo = b & np.uint32(0xFFF)
    hi = b & ~np.uint32(0xFFF)
    rup = (lo > 0x800) | ((lo == 0x800) & (((hi >> 12) & 1) == 1))
    return (hi + (rup.astype(np.uint32) << 12)).view(np.float32)


def _build_program(nsets, reps=1):
    """Build + compile the SPMD device program.

    nsets: number of 512-wide kvec column groups (1 for the symmetric-half
    fast path, 2 for the full-set fallback).
    reps: emit the whole computation `reps` times (benchmark delta timing).
    """
    AluOp = mybir.AluOpType
    AF = mybir.ActivationFunctionType
    NKP = 512 * nsets

    nc = bacc.Bacc("TRN2", target_bir_lowering=False, debug=False,
                   num_devices=N_CORES)

    def din(name, shape, dt=F32):
        return nc.dram_tensor(name, shape, dt, kind="ExternalInput").ap()

    def dout(name, shape):
        return nc.dram_tensor(name, shape, F32, kind="ExternalOutput").ap()

    u_hi = din("u_hi", [NSUP, 3, 256], F32R)   # fractional coords, hi part
    u_lo = din("u_lo", [NSUP, 3, 256], F32R)   # fractional coords, lo part
    kv_t = din("kv_t", [3, NKP], F32R)         # kvec rows, padded
    one_t = din("one_t", [128, 1], F32R)       # ones column (fp32r memset
    #                                            is rejected by codegen)
    qoh = din("qoh", [NSUP, 128, 128], F32R)   # q-scaled one-hot
    ds = din("ds", [128, CCOLS], F16)          # pair distance d (fp16)
    qq = din("qq", [128, CCOLS], F16)          # q[i]*q[j] per pair (fp16)

    o_qr = dout("o_qr", [64, NKP])         # sum q*cos per (mol, kvec)
    o_qi = dout("o_qi", [64, NKP])         # sum q*sin
    o_cs = dout("o_cs", [1, CCOLS])        # per-column pair-potential sums

    with tile.TileContext(nc, trace_sim=False) as tc, ExitStack() as ctx:
        pers = ctx.enter_context(tc.tile_pool(name="pers", bufs=1))
        io = ctx.enter_context(tc.tile_pool(name="io", bufs=3))
        tmp = ctx.enter_context(tc.tile_pool(name="tmp", bufs=2))
        rtmp = ctx.enter_context(tc.tile_pool(name="rtmp", bufs=2))
        ps_t = ctx.enter_context(
            tc.tile_pool(name="ps_t", bufs=2 if nsets == 1 else 1,
                         space="PSUM"))
        ps_acc = ctx.enter_context(
            tc.tile_pool(name="ps_acc", bufs=1, space="PSUM"))
        ps_cs = ctx.enter_context(
            tc.tile_pool(name="ps_cs", bufs=1, space="PSUM"))

        # persistent SBUF
        kv_sb = pers.tile([3, NKP], F32R)
        ones = pers.tile([128, 1], F32R)
        halfpi = pers.tile([128, 1], F32)
        negq = pers.tile([128, 1], F32)
        qr_sb = pers.tile([64, NKP], F32)
        qi_sb = pers.tile([64, NKP], F32)

        nc.vector.memset(halfpi[:], math.pi / 2)
        nc.vector.memset(negq[:], -0.25)
        nc.sync.dma_start(ones[:], one_t[:])
        nc.sync.dma_start(kv_sb[:], kv_t[:])

        def _emit_rc_tile(i):
            # one real-space tile: stream d/qq, pair potential, column reduce
            sl = slice(i * CT, (i + 1) * CT)
            dt_ = io.tile([128, CT], F16, tag="dt")
            nc.sync.dma_start(dt_[:], ds[:, sl])
            qqt = io.tile([128, CT], F16, tag="qq")
            nc.sync.dma_start(qqt[:], qq[:, sl])
            ri = rtmp.tile([128, CT], F32, tag="ri")
            nc.vector.reciprocal(ri[:], dt_[:])
            e = rtmp.tile([128, CT], F32, tag="e")
            nc.scalar.activation(e[:], dt_[:], AF.Erf, scale=-SQA)
            f = rtmp.tile([128, CT], F32, tag="f")
            # (1 - erf(sqa d)) * (1/d)
            nc.vector.scalar_tensor_tensor(f[:], e[:], 1.0, ri[:],
                                           AluOp.add, AluOp.mult)
            # cutoff: f(d) is strictly decreasing, so d<=CUTOFF is
            # exactly f-FCUT>=0 -> relu replaces the compare+select
            p = rtmp.tile([128, CT], F32, tag="p")
            nc.vector.tensor_scalar(p[:], f[:], FCUT, 0.0,
                                    AluOp.subtract, AluOp.max)
            pot = rtmp.tile([128, CT], F32R, tag="pot")
            # Pool only supports tensor_tensor-class ops; give it this one
            nc.gpsimd.tensor_tensor(pot[:], p[:], qqt[:], AluOp.mult)
            for j in range(CT // 512):
                cps = ps_cs.tile([1, 512], F32, tag="cs")
                nc.tensor.matmul(cps[:], ones[:],
                                 pot[:, j * 512:(j + 1) * 512],
                                 start=True, stop=True)
                cs_sb = io.tile([1, 512], F32, tag="cso")
                nc.vector.tensor_copy(cs_sb[:], cps[:])
                lo = i * CT + j * 512
                nc.sync.dma_start(o_cs[0:1, lo:lo + 512], cs_sb[:])

        def _emit_once():
            # ---- Phase K: reciprocal-space phases + segment sums,
            # with the real-space tiles interleaved ----
            qr_ps = ps_acc.tile([64, NKP], F32, tag="qr")
            qi_ps = ps_acc.tile([64, NKP], F32, tag="qi")
            for s in range(NSUP):
                if s % 7 == 3:
                    _emit_rc_tile(s // 7)
                qoh_t = io.tile([128, 128], F32R, tag="qoh")
                nc.sync.dma_start(qoh_t[:], qoh[s])
                ut_hi = io.tile([3, 256], F32R, tag="uth")
                nc.sync.dma_start(ut_hi[:], u_hi[s])
                ut_lo = io.tile([3, 256], F32R, tag="utl")
                nc.sync.dma_start(ut_lo[:], u_lo[s])
                for kset in range(nsets):
                    ksl = slice(kset * 512, (kset + 1) * 512)
                    tt = ps_t.tile([128, 1024], F32, tag="tt")
                    for h in range(2):
                        # u = u_hi + u_lo: two fp32r matmuls accumulate the
                        # full-precision phase into PSUM
                        nc.tensor.matmul(
                            tt[:, h * 512:(h + 1) * 512],
                            ut_hi[:, h * 128:(h + 1) * 128],
                            kv_sb[:, ksl], start=True, stop=False)
                        nc.tensor.matmul(
                            tt[:, h * 512:(h + 1) * 512],
                            ut_lo[:, h * 128:(h + 1) * 128],
                            kv_sb[:, ksl], start=False, stop=True)
                    # range reduction: g = t - round(t) in [-0.5, 0.5]
                    rnd = tmp.tile([128, 1024], F32, tag="rnd")
                    nc.vector.tensor_scalar(rnd[:], tt[:], MAGIC, MAGIC,
                                            AluOp.add, AluOp.subtract)
                    # wga = [0.25 - g | |g|]: both halves through ONE Sin with
                    # scale=-2pi bias=pi/2 give sin(2pi t) and cos(2pi t).
                    wga = tmp.tile([128, 2048], F32, tag="wga")
                    w = wga[:, 0:1024]
                    nc.vector.scalar_tensor_tensor(w, rnd[:], 0.25, tt[:],
                                                   AluOp.add, AluOp.subtract)
                    # |g| = |w - 0.25| on ACT (Abs is in every table set)
                    nc.scalar.activation(wga[:, 1024:2048], w, AF.Abs,
                                         bias=negq[:])
                    cs_t = tmp.tile([128, 2048], F32R, tag="cs")
                    nc.scalar.activation(cs_t[:], wga[:], AF.Sin,
                                         scale=-TWO_PI, bias=halfpi[:])
                    for h in range(2):
                        ch = 2 * s + h
                        lhs = qoh_t[:, h * 64:(h + 1) * 64]
                        first = (ch == 0)
                        last = (ch == ACH - 1)
                        nc.tensor.matmul(qr_ps[:, ksl], lhs,
                                         cs_t[:, 1024 + h * 512:
                                              1536 + h * 512],
                                         start=first, stop=last,
                                         skip_group_check=True)
                        nc.tensor.matmul(qi_ps[:, ksl], lhs,
                                         cs_t[:, h * 512:512 + h * 512],
                                         start=first, stop=last,
                                         skip_group_check=True)

            # ---- finale: copy accumulators out ----
            nc.vector.tensor_copy(qr_sb[:], qr_ps[:])
            nc.vector.tensor_copy(qi_sb[:], qi_ps[:])
            nc.sync.dma_start(o_qr[:], qr_sb[:])
            nc.sync.dma_start(o_qi[:], qi_sb[:])

        for _rep in range(reps):
            _emit_once()

    nc.compile()
    return nc


def _get_program(nsets, reps=1):
    key = (nsets, reps)
    if key not in _PROG_CACHE:
        _PROG_CACHE[key] = _build_program(nsets, reps)
    return _PROG_CACHE[key]


def _half_kvecs(kvecs):
    """Pick one of each +-k pair.  Returns selected row indices, or None if
    the set is not exactly +-symmetric."""
    nk = kvecs.shape[0]
    key = {tuple(v): i for i, v in enumerate(kvecs)}
    partner = np.full(nk, -1, np.int64)
    for i, v in enumerate(kvecs):
        j = key.get(tuple(-v))
        if j is None:
            return None
        partner[i] = j
    if np.any(partner == np.arange(nk)):
        return None  # self-negative (k=0) unsupported here
    sel = np.where(np.arange(nk) < partner)[0]
    if sel.size * 2 != nk:
        return None
    return sel


def prepare(inputs):
    """Host prep: returns (nc, in_maps, combine_fn)."""
    q = np.asarray(inputs["partial_charges"], np.float32)[:, 0]
    Rij = np.asarray(inputs["Rij"], np.float32)
    R = np.asarray(inputs["R"], np.float32)
    cell = np.asarray(inputs["cell"], np.float32)
    kvecs = np.asarray(inputs["kvecs"], np.float32)
    idx_m = np.asarray(inputs["idx_m"]).astype(np.int64)
    idx_i = np.asarray(inputs["idx_i"]).astype(np.int64)
    idx_j = np.asarray(inputs["idx_j"]).astype(np.int64)

    sel = _half_kvecs(kvecs)
    if sel is not None:
        kv_use = kvecs[sel]
        wk = 2.0
    else:
        kv_use = kvecs
        wk = 1.0
    nkh = kv_use.shape[0]
    nsets = (nkh + 511) // 512
    NKP = 512 * nsets
    nc = _get_program(nsets)

    # ---------- host prep: reciprocal space ----------
    invc = np.linalg.inv(cell.astype(np.float64))
    u_all = np.einsum("ae,aed->ad", R, invc[idx_m]).astype(np.float32)

    kv_t_np = np.zeros((3, NKP), np.float32)
    kv_t_np[:, :nkh] = to_f32r(kv_use.T)

    # ---------- host prep: real space ----------
    mol_pair = idx_m[idx_i].astype(np.int32)
    qq_pair = q[idx_i] * q[idx_j]
    d_pair = np.sqrt(np.einsum("pd,pd->p", Rij, Rij)).astype(np.float32)
    order = np.argsort(mol_pair, kind="stable")
    d_s = d_pair[order]
    qq_s = qq_pair[order]
    counts = np.bincount(mol_pair, minlength=N_MOL)
    starts = np.concatenate(([0], np.cumsum(counts)))

    in_maps = []
    colmols = []
    SLOTS = CCOLS * 128
    for c in range(N_CORES):
        gidx = np.full(SLOTS, -1, np.int64)   # [p, col] flattened p*CCOLS+col
        colmol = np.full(CCOLS, -1, np.int32)
        col0 = 0
        for m in range(N_MOL):
            n = counts[m]
            share = (n + N_CORES - 1) // N_CORES
            lo = starts[m] + c * share
            hi = min(starts[m] + n, lo + share)
            ncm = max(hi - lo, 0)
            if ncm == 0:
                continue
            ncols = (ncm + 127) // 128
            js = np.arange(ncm)
            gidx[(js % 128) * CCOLS + col0 + js // 128] = lo + js
            colmol[col0:col0 + ncols] = m
            col0 += ncols
        assert col0 <= CCOLS, f"column overflow: {col0} > {CCOLS}"
        valid = gidx >= 0
        gv = gidx[valid]

        def fill(src, pad):
            a = np.full(SLOTS, pad, np.float32)
            a[valid] = src[gv]
            return a.reshape(128, CCOLS)

        # atoms for this core: round-robin slice keeps mol-sorted order
        a_ids = np.arange(c, N_ATOMS, N_CORES)
        u_core = np.zeros((APAD, 3), np.float32)
        u_core[:APC] = u_all[a_ids]
        q_core = np.zeros(APAD, np.float32)
        q_core[:APC] = q[a_ids]
        m_core = np.zeros(APAD, np.int64)
        m_core[:APC] = idx_m[a_ids]
        qoh_np = np.zeros((APAD, 64), np.float32)
        qoh_np[np.arange(APAD), m_core] = to_f32r(q_core)
        qoh_np = qoh_np.reshape(NSUP, 2, 128, 64).transpose(0, 2, 1, 3) \
                       .reshape(NSUP, 128, 128)
        # u = u_hi + u_lo, both on the fp32r grid, summing exactly to u
        u_hi_core = to_f32r(u_core)
        u_lo_core = to_f32r(u_core - u_hi_core)
        u_hi_np = np.ascontiguousarray(
            u_hi_core.reshape(NSUP, 256, 3).transpose(0, 2, 1))
        u_lo_np = np.ascontiguousarray(
            u_lo_core.reshape(NSUP, 256, 3).transpose(0, 2, 1))

        in_maps.append({
            "u_hi": u_hi_np,
            "u_lo": u_lo_np,
            "kv_t": kv_t_np,
            "one_t": np.ones((128, 1), np.float32),
            "qoh": np.ascontiguousarray(qoh_np),
            "ds": fill(d_s, PAD_X).astype(np.float16),
            "qq": fill(qq_s, 0.0).astype(np.float16),
        })
        colmols.append(colmol)

    self_q2_host = np.bincount(idx_m, weights=(q.astype(np.float64) ** 2),
                               minlength=N_MOL)

    def combine(results):
        q_real = np.zeros((64, nkh), np.float64)
        q_imag = np.zeros((64, nkh), np.float64)
        self_q2 = self_q2_host
        y_real = np.zeros(64, np.float64)
        for c in range(N_CORES):
            out = results[c]
            q_real += out["o_qr"][:, :nkh]
            q_imag += out["o_qi"][:, :nkh]
            cs = out["o_cs"][0]
            cm = colmols[c]
            used = cm >= 0
            y_real += np.bincount(cm[used], weights=cs[used], minlength=64)

        # O(M*K) tail math (float64 on host, cast at the end)
        recip = TWO_PI * np.transpose(invc, (0, 2, 1))     # [M,3,3]
        v_box = np.abs(np.linalg.det(cell.astype(np.float64)))
        prefactor = TWO_PI / v_box
        kv_m = np.einsum("kd,mde->mke", kv_use.astype(np.float64), recip)
        k_sq = np.sum(kv_m ** 2, axis=2)                   # [M,Kh]
        q_gauss = np.exp(-0.25 * k_sq / ALPHA)
        q_dens = q_real ** 2 + q_imag ** 2
        y_ewald = prefactor * np.sum(wk * q_dens * q_gauss / k_sq, axis=1)
        self_int = math.sqrt(ALPHA / math.pi) * self_q2
        y = 0.5 * KE * y_real + KE * (y_ewald - self_int)
        return y.astype(np.float32)

    return nc, in_maps, combine


def kernel(**inputs):
    nc, in_maps, combine = prepare(inputs)
    res = bass_utils.run_bass_kernel_spmd(nc, in_maps,
                                          core_ids=list(range(N_CORES)))
    return combine(res.results)
